# revision 53
# baseline (speedup 1.0000x reference)
"""Trainium2 Bass kernel for nn_AttentionLayer (sparse_attention).

Math per batch b (8 batches -> 8 cores, data parallel):
  q = Wq @ x, k = Wk @ x, v = Wv @ x            (x as [C=768, HW=4096])
  Qf/Kf/Vf = [L=6144, W=64]   (row index l = c*64 + h)
  S = Qf @ Kf^T  [6144, 6144]; beta = softmax(S, axis=-1)
  attn = beta @ Vf; out = gamma * Wc @ attn(as [96,4096]) + x

Kernel strategy (per core), v2:
  - Reorder both query and key/value index as l' = h*96 + c (softmax over l is
    permutation invariant) so layout changes are contiguous copies.
  - QKV 1x1 convs run as float32r matmuls straight off the DMAed x tiles
    (full PE rate at >=256 moving columns) - no bf16 cast of x at all.
  - Q/K stored fp8e4 in DoubleRow-packed layout [64(w), 2, 1024+1024] per
    l'-group (plane 1 zeroed): S^T chunk matmuls run at 0.5 cycles/col.
  - exp split across ScalarE (true Exp) and DVE (Schraudolph bit trick:
    int16(A*s+B) viewed as bf16 approximates e^s to ~2-3%, far inside this
    layer's error budget - the attention term is ~0.2% of the output norm).
  - PV uses the transposed dataflow: lhsT = es chunk [128 l', 128 ch'],
    rhs = vf [128 l', 65] (col 64 = ones -> softmax denominator), PSUM
    accumulators [128 ch', 65] per pass: half the streamed columns of the
    vf-stationary dataflow, and attn^T comes out pre-transposed for the
    final projection (no PE transposes).
  - vf chunks produced by DMA-transpose (SBUF->SBUF xbar), freeing PE/DVE.
  - Normalization reads attn^T h-block segments [<=96, 64] straight out of
    PSUM with a per-pass reciprocal of the denominator column; segments are
    emitted as soon as their pass finishes so the single PV PSUM slot can
    recycle.
"""

import os
from contextlib import ExitStack

import ml_dtypes
import numpy as np

import concourse.bass as bass
import concourse.mybir as mybir
import concourse.tile as tile
from concourse import bacc
from concourse.bass_utils import run_bass_kernel_spmd
from concourse.masks import make_identity

C = 768
CK = 96
H = 64
W = 64
HW = H * W          # 4096
L = CK * H          # 6144
NCHUNK = L // 128   # 48
FP = mybir.dt.float32
FR = mybir.dt.float32r
BF = mybir.dt.bfloat16
F8 = mybir.dt.float8e4
I16 = mybir.dt.int16

# Schraudolph exp: bf16 bits of e^s ~= int16(round(A*s + B))
EXP_A = 128.0 / float(np.log(2.0))
EXP_B = 16250.0

# fraction of exp chunks handled by ScalarE (true Exp); DVE (Schraudolph)
# takes the rest, interleaved chunk by chunk
EXP_ACT = 28.0 / 48.0

_CACHE = {}



def _build_program():
    nc = bacc.Bacc(
        "TRN2",
        target_bir_lowering=False,
        debug=False,
        enable_asserts=False,
        num_devices=8,
    )
    x = nc.dram_tensor("x", [C, HW], FP, kind="ExternalInput").ap()
    wt = nc.dram_tensor("wt", [C, 3 * CK], BF, kind="ExternalInput").ap()
    wct = nc.dram_tensor("wct", [CK, C], BF, kind="ExternalInput").ap()
    out = nc.dram_tensor("out", [C, HW], FP, kind="ExternalOutput").ap()

    x_t = x.rearrange("(k p) f -> k p f", p=128)      # [6, 128, 4096]
    out_t = out.rearrange("(k p) f -> k p f", p=128)  # [6, 128, 4096]

    mm = mybir.AluOpType.mult
    ad = mybir.AluOpType.add
    DR = mybir.MatmulPerfMode.DoubleRow

    with ExitStack() as ctx:
        tc = ctx.enter_context(tile.TileContext(nc))
        singles = ctx.enter_context(tc.tile_pool(name="singles", bufs=1))
        xpool = ctx.enter_context(tc.tile_pool(name="xpool", bufs=2))
        expp = ctx.enter_context(tc.tile_pool(name="expp", bufs=8))
        outp = ctx.enter_context(tc.tile_pool(name="outp", bufs=2))
        usb = ctx.enter_context(tc.tile_pool(name="usb", bufs=2))
        rp = ctx.enter_context(tc.tile_pool(name="rp", bufs=3))
        # PSUM: 8 banks total. sp needs 3 bufs (2 banks each) so the S->exp
        # WAR turnaround hides behind two in-flight exps; un gets the other
        # 2 banks. Stage-3 transposes and projection tiles borrow sp-ring
        # slots at pass boundaries.
        ps_s = ctx.enter_context(tc.tile_pool(name="ps_s", bufs=3, space="PSUM"))
        ps_u = ctx.enter_context(tc.tile_pool(name="ps_u", bufs=1, space="PSUM"))

        # ---- constants ----
        ident = singles.tile([65, 65], FP)
        make_identity(nc, ident)
        wt_sb = singles.tile([128, 6, 3 * CK], BF)
        nc.sync.dma_start(out=wt_sb, in_=wt.rearrange("(k p) n -> p k n", p=128))
        wct_sb = singles.tile([CK, C], BF)
        nc.sync.dma_start(out=wct_sb, in_=wct)

        # q/k packed for DoubleRow: per l'-group t of 1024: [64(w), 2, 2048]
        # (plane 1 = zeros; cols 0:1024 = q of ch'-group t, 1024:2048 = k of
        # l'-group t), fp8e4.
        qk_tiles = [
            singles.tile([64, 2, 2048], F8, tag=f"qk{t}", name=f"qk{t}")
            for t in range(6)
        ]
        for t in range(6):
            if t % 2 == 0:
                nc.gpsimd.memset(qk_tiles[t][:, 1, :], 0.0)
            else:
                nc.vector.memset(qk_tiles[t][:, 1, :], 0.0)
        # v in w-major layout [64, 6144] bf16 (transposed into vf chunks)
        vw = singles.tile([64, L], BF, name="vw")
        vf_tiles = [
            singles.tile([128, 65], BF, tag=f"vf{t}", name=f"vf{t}")
            for t in range(NCHUNK)
        ]

        attn_tiles = [
            singles.tile([CK, 512], BF, tag=f"attn{jj}", name=f"attn{jj}")
            for jj in range(8)
        ]
        # x kept resident in bf16: QKV matmul input + stage-4 residual
        xb_tiles = [
            singles.tile([128, 6, 512], BF, tag=f"xb{hb}", name=f"xb{hb}")
            for hb in range(8)
        ]

        def qk_evac(h, src):
            """Copy [64, 192] q|k block (fp32 psum) into the packed fp8
            tiles: q -> group (h*96)//1024 col off, k -> same group at
            1024+off; splits when an h-block crosses a 1024 boundary."""
            p0 = (h * CK) // 1024
            off = h * CK - p0 * 1024

            def one(t, o, cs, cn):
                dst = qk_tiles[t][:, 0, :].rearrange("p (qk n) -> p qk n", qk=2)
                nc.vector.tensor_copy(
                    out=dst[:, :, o:o + cn],
                    in_=src.rearrange("p (qk n) -> p qk n", qk=2)[:, :, cs:cs + cn],
                )

            if off + CK <= 1024:
                one(p0, off, 0, CK)
            else:
                f1 = 1024 - off
                one(p0, off, 0, f1)
                one(p0 + 1, 0, f1, CK - f1)

        # ---- stage 1: QKV in w-major layout ----
        # x loads go through the Pool-triggered SWDGE queue so they never
        # queue behind the 48 vf DMA-transposes on the SP/HWDGE path; the two
        # half tiles + per-k-chunk casts (ScalarE/Pool alternating) unblock
        # the QKV matmuls after ~1/6 of the cast.
        for hb in range(8):  # h in batches of 8
            xch = [
                xpool.tile([128, 3, 512], FP, tag=f"xc{dh}", name=f"xc{hb}_{dh}")
                for dh in range(2)
            ]
            for dh in range(2):
                nc.sync.dma_start(
                    out=xch[dh],
                    in_=x_t[dh * 3:(dh + 1) * 3, :,
                            hb * 512:(hb + 1) * 512].rearrange("k p f -> p k f"),
                )
            xb = xb_tiles[hb]
            for kc in range(6):
                if kc % 2 == 0:
                    nc.scalar.copy(xb[:, kc, :], xch[kc // 3][:, kc % 3, :])
                else:
                    nc.gpsimd.tensor_copy(
                        out=xb[:, kc, :], in_=xch[kc // 3][:, kc % 3, :]
                    )
            for hp in range(4):
                qp = ps_s.tile([128, 3 * CK], FP, tag="s")
                for kc in range(6):
                    nc.tensor.matmul(
                        qp,
                        xb[:, kc, hp * 128:(hp + 1) * 128],
                        wt_sb[:, kc, :],
                        start=(kc == 0),
                        stop=(kc == 5),
                    )
                for sub in range(2):
                    h = hb * 8 + hp * 2 + sub
                    qs = qp[sub * 64:(sub + 1) * 64, :]
                    qk_evac(h, qs[:, 0:2 * CK])
                    eng = nc.scalar.copy if h % 2 else (
                        lambda o, i: nc.vector.tensor_copy(out=o, in_=i)
                    )
                    eng(vw[:, h * CK:(h + 1) * CK], qs[:, 2 * CK:3 * CK])

        # vf chunks via SBUF->SBUF DMA transpose, emitted after all x loads so
        # they never delay stage 1 on the single in-order HWDGE pipe; they
        # drain during stage 2's ramp, just ahead of the PV consumers.
        for t in range(NCHUNK):
            nc.gpsimd.memset(vf_tiles[t][:, 64:65], 1.0)  # denominator ones
            nc.sync.dma_start_transpose(
                out=vf_tiles[t][:, 0:64],
                in_=vw[:, t * 128:(t + 1) * 128],
            )

        # ---- stage 2 + interleaved stage 3/4 ----
        unsb_tiles = {}

        # h-block h (96 attn^T rows) is ready once pass p covers its last
        # column: (h+1)*96 <= 1024*(p+1). Boundary blocks gather from the two
        # SBUF-staged un tiles (usb pool keeps the previous pass alive).
        ready_by_pass = {p: [] for p in range(6)}
        for h in range(64):
            ready_by_pass[((h + 1) * CK - 1) // 1024].append(h)
        jj_after = {p: [] for p in range(6)}
        cnt = [0] * 8
        for p in range(6):
            for h in ready_by_pass[p]:
                cnt[h // 8] += 1
            for jj in range(8):
                if cnt[jj] == 8:
                    jj_after[p].append(jj)
                    cnt[jj] = -(1 << 30)  # emitted

        def emit_h(p, h):
            jj, hl = h // 8, h % 8
            off = h * CK - p * 1024
            if off >= 0:
                src = unsb_tiles[p][:, off:off + CK]
            else:
                f1 = -off  # cols in previous pass
                st = rp.tile([65, CK], FP, tag="st")
                nc.vector.tensor_copy(
                    out=st[:, 0:f1], in_=unsb_tiles[p - 1][:, 1024 - f1:1024]
                )
                nc.vector.tensor_copy(
                    out=st[:, f1:CK], in_=unsb_tiles[p][:, 0:CK - f1]
                )
                src = st
            tp = ps_s.tile([CK, 65], FP, tag="s")
            nc.tensor.transpose(tp, src, ident)
            r = rp.tile([CK, 1], FP, tag="r")
            nc.vector.reciprocal(r, tp[:, 64:65])
            nc.scalar.activation(
                attn_tiles[jj][:, hl * 64:(hl + 1) * 64], tp[:, 0:64],
                mybir.ActivationFunctionType.Copy, scale=r,
            )

        ob_tiles = {}

        def emit_co(jj, co):
            if co == 0:
                ob_tiles[jj] = outp.tile(
                    [128, 6, 512], FP, tag="ob", name=f"ob{jj}"
                )
            op = ps_s.tile([128, 512], FP, tag="s")
            nc.tensor.matmul(
                op,
                wct_sb[:, co * 128:(co + 1) * 128],
                attn_tiles[jj],
                start=True,
                stop=True,
            )
            nc.vector.tensor_add(
                ob_tiles[jj][:, co, :], op, xb_tiles[jj][:, co, :]
            )
            if co == 5:
                nc.sync.dma_start(
                    out=out_t[:, :, jj * 512:(jj + 1) * 512].rearrange(
                        "k p f -> p k f"
                    ),
                    in_=ob_tiles[jj],
                )

        # All boundary work (un evacuation, normalization, projection) is
        # injected into the NEXT pass's chunk loop once its S/exp pipeline is
        # rolling; emitting it between passes would block the sp ring and
        # idle both exp engines for several us per boundary.
        def emit_unsb(p, un):
            unsb = usb.tile([65, 1024], FP, tag="unsb")
            unsb_tiles[p] = unsb
            if p % 2 == 0:
                nc.scalar.copy(unsb, un)
            else:
                nc.vector.tensor_copy(out=unsb, in_=un)

        # exp engine per chunk: ScalarE (true Exp) for EXP_ACT of every
        # chunk on average, DVE (Schraudolph) for the rest, interleaved
        inject = []  # [(lt_trigger, fn), ...] boundary work for prev pass
        acc = 0.0
        for p in range(6):
            un = ps_u.tile([65, 1024], FP, tag="un")

            def emit_pv(lt, es, un=un):
                for j in range(2):
                    nc.tensor.matmul(
                        un[:, j * 512:(j + 1) * 512],
                        vf_tiles[lt],
                        es[:, j * 512:(j + 1) * 512],
                        start=(lt == 0),
                        stop=(lt == NCHUNK - 1),
                    )

            # Software-pipelined emission: PE executes its queue in order, so
            # PV(lt) (which waits on exp(lt)) is emitted only after S(lt+2).
            # Depth 2 is required: with depth 1 the stream [PV(lt-1), S(lt)]
            # stalls PE on exp(lt-1), which serializes consecutive exps and
            # idles both exp engines half the time.
            pend = []
            for lt in range(NCHUNK):
                sp = ps_s.tile([128, 1024], FP, tag="s")
                for j in range(2):
                    nc.tensor.matmul(
                        sp[:, j * 512:(j + 1) * 512],
                        qk_tiles[lt // 8][:, :, 1024 + (lt % 8) * 128:1024 + (lt % 8 + 1) * 128],
                        qk_tiles[p][:, :, j * 512:(j + 1) * 512],
                        start=True,
                        stop=True,
                        perf_mode=DR,
                    )
                es = expp.tile([128, 1024], BF, tag="es")
                acc += EXP_ACT
                if acc >= 1.0:
                    acc -= 1.0
                    nc.scalar.activation(
                        es, sp, mybir.ActivationFunctionType.Exp
                    )
                else:
                    nc.vector.tensor_scalar(
                        es[:, :].bitcast(I16), sp, EXP_A, EXP_B, mm, ad
                    )
                pend.append((lt, es))
                if len(pend) > 2:
                    emit_pv(*pend.pop(0))
                for trig, fn in inject:
                    if trig == lt:
                        fn()
            for pe in pend:
                emit_pv(*pe)
            # schedule the boundary work spread across the next pass's chunks
            # (one item every other chunk) so the sp ring always keeps S tiles
            # in flight
            inject = [(0, lambda p=p, un=un: emit_unsb(p, un))]
            slot = 2
            for h in ready_by_pass[p]:
                inject.append((slot, lambda p=p, h=h: emit_h(p, h)))
                slot += 1
            slot += 1
            for jj in jj_after[p]:
                for co in range(6):
                    inject.append(
                        (slot, lambda jj=jj, co=co: emit_co(jj, co))
                    )
                    slot += 2
        for _, fn in inject:
            fn()

    nc.finalize()
    return nc


def _get_program():
    if "nc" not in _CACHE:
        _CACHE["nc"] = _build_program()
    return _CACHE["nc"]


def _host_weights(Wq, Wk, Wv, Wc, gamma):
    wt_host = np.ascontiguousarray(
        np.concatenate([Wq.T, Wk.T, Wv.T], axis=1)
    ).astype(ml_dtypes.bfloat16)                       # [768, 288]
    wct_host = np.ascontiguousarray((gamma[0] * Wc).T).astype(
        ml_dtypes.bfloat16
    )                                                  # [96, 768]
    return wt_host, wct_host


def kernel(x, Wq, Wk, Wv, Wc, gamma):
    x = np.asarray(x, dtype=np.float32)
    Wq = np.asarray(Wq, dtype=np.float32)
    Wk = np.asarray(Wk, dtype=np.float32)
    Wv = np.asarray(Wv, dtype=np.float32)
    Wc = np.asarray(Wc, dtype=np.float32)
    gamma = np.asarray(gamma, dtype=np.float32)

    B = x.shape[0]
    assert x.shape == (B, C, H, W) and B == 8

    wt_host, wct_host = _host_weights(Wq, Wk, Wv, Wc, gamma)
    in_maps = [
        {
            "x": np.ascontiguousarray(x[b].reshape(C, HW)),
            "wt": wt_host,
            "wct": wct_host,
        }
        for b in range(B)
    ]

    nc = _get_program()
    trace = os.environ.get("KERNEL_TRACE", "0") == "1"
    res = run_bass_kernel_spmd(
        nc, in_maps, core_ids=list(range(8)), trace=trace
    )
    if trace and res.exec_time_ns is not None:
        print(f"HW exec time: {res.exec_time_ns} ns")
        _CACHE["exec_time_ns"] = res.exec_time_ns

    out = np.stack([r["out"].reshape(C, H, W) for r in res.results])
    return out


# revision 54
# speedup vs baseline: 1.0241x; 1.0241x over previous
"""Trainium2 Bass kernel for nn_AttentionLayer (sparse_attention).

Math per batch b (8 batches -> 8 cores, data parallel):
  q = Wq @ x, k = Wk @ x, v = Wv @ x            (x as [C=768, HW=4096])
  Qf/Kf/Vf = [L=6144, W=64]   (row index l = c*64 + h)
  S = Qf @ Kf^T  [6144, 6144]; beta = softmax(S, axis=-1)
  attn = beta @ Vf; out = gamma * Wc @ attn(as [96,4096]) + x

Kernel strategy (per core), v2:
  - Reorder both query and key/value index as l' = h*96 + c (softmax over l is
    permutation invariant) so layout changes are contiguous copies.
  - QKV 1x1 convs run as float32r matmuls straight off the DMAed x tiles
    (full PE rate at >=256 moving columns) - no bf16 cast of x at all.
  - Q/K stored fp8e4 in DoubleRow-packed layout [64(w), 2, 1024+1024] per
    l'-group (plane 1 zeroed): S^T chunk matmuls run at 0.5 cycles/col.
  - exp split across ScalarE (true Exp) and DVE (Schraudolph bit trick:
    int16(A*s+B) viewed as bf16 approximates e^s to ~2-3%, far inside this
    layer's error budget - the attention term is ~0.2% of the output norm).
  - PV uses the transposed dataflow: lhsT = es chunk [128 l', 128 ch'],
    rhs = vf [128 l', 65] (col 64 = ones -> softmax denominator), PSUM
    accumulators [128 ch', 65] per pass: half the streamed columns of the
    vf-stationary dataflow, and attn^T comes out pre-transposed for the
    final projection (no PE transposes).
  - vf chunks produced by DMA-transpose (SBUF->SBUF xbar), freeing PE/DVE.
  - Normalization reads attn^T h-block segments [<=96, 64] straight out of
    PSUM with a per-pass reciprocal of the denominator column; segments are
    emitted as soon as their pass finishes so the single PV PSUM slot can
    recycle.
"""

import os
from contextlib import ExitStack

import ml_dtypes
import numpy as np

import concourse.bass as bass
import concourse.mybir as mybir
import concourse.tile as tile
from concourse import bacc
from concourse.bass_utils import run_bass_kernel_spmd
from concourse.masks import make_identity

C = 768
CK = 96
H = 64
W = 64
HW = H * W          # 4096
L = CK * H          # 6144
NCHUNK = L // 128   # 48
FP = mybir.dt.float32
FR = mybir.dt.float32r
BF = mybir.dt.bfloat16
F8 = mybir.dt.float8e4
I16 = mybir.dt.int16

# Schraudolph exp: bf16 bits of e^s ~= int16(round(A*s + B))
EXP_A = 128.0 / float(np.log(2.0))
EXP_B = 16250.0

# fraction of exp chunks handled by ScalarE (true Exp); DVE (Schraudolph)
# takes the rest, interleaved chunk by chunk
EXP_ACT = 27.0 / 48.0

_CACHE = {}



def _build_program():
    nc = bacc.Bacc(
        "TRN2",
        target_bir_lowering=False,
        debug=False,
        enable_asserts=False,
        num_devices=8,
    )
    x = nc.dram_tensor("x", [C, HW], FP, kind="ExternalInput").ap()
    wt = nc.dram_tensor("wt", [C, 3 * CK], BF, kind="ExternalInput").ap()
    wct = nc.dram_tensor("wct", [CK, C], BF, kind="ExternalInput").ap()
    out = nc.dram_tensor("out", [C, HW], FP, kind="ExternalOutput").ap()

    x_t = x.rearrange("(k p) f -> k p f", p=128)      # [6, 128, 4096]
    out_t = out.rearrange("(k p) f -> k p f", p=128)  # [6, 128, 4096]

    mm = mybir.AluOpType.mult
    ad = mybir.AluOpType.add
    DR = mybir.MatmulPerfMode.DoubleRow

    with ExitStack() as ctx:
        tc = ctx.enter_context(tile.TileContext(nc))
        singles = ctx.enter_context(tc.tile_pool(name="singles", bufs=1))
        xpool = ctx.enter_context(tc.tile_pool(name="xpool", bufs=2))
        expp = ctx.enter_context(tc.tile_pool(name="expp", bufs=8))
        outp = ctx.enter_context(tc.tile_pool(name="outp", bufs=2))
        usb = ctx.enter_context(tc.tile_pool(name="usb", bufs=2))
        rp = ctx.enter_context(tc.tile_pool(name="rp", bufs=3))
        # PSUM: 8 banks total. sp needs 3 bufs (2 banks each) so the S->exp
        # WAR turnaround hides behind two in-flight exps; un gets the other
        # 2 banks. Stage-3 transposes and projection tiles borrow sp-ring
        # slots at pass boundaries.
        ps_s = ctx.enter_context(tc.tile_pool(name="ps_s", bufs=3, space="PSUM"))
        ps_u = ctx.enter_context(tc.tile_pool(name="ps_u", bufs=1, space="PSUM"))

        # ---- constants ----
        ident = singles.tile([65, 65], FP)
        make_identity(nc, ident)
        wt_sb = singles.tile([128, 6, 3 * CK], BF)
        nc.sync.dma_start(out=wt_sb, in_=wt.rearrange("(k p) n -> p k n", p=128))
        wct_sb = singles.tile([CK, C], BF)
        nc.sync.dma_start(out=wct_sb, in_=wct)

        # q/k packed for DoubleRow: per l'-group t of 1024: [64(w), 2, 2048]
        # (plane 1 = zeros; cols 0:1024 = q of ch'-group t, 1024:2048 = k of
        # l'-group t), fp8e4.
        qk_tiles = [
            singles.tile([64, 2, 2048], F8, tag=f"qk{t}", name=f"qk{t}")
            for t in range(6)
        ]
        for t in range(6):
            if t % 2 == 0:
                nc.gpsimd.memset(qk_tiles[t][:, 1, :], 0.0)
            else:
                nc.vector.memset(qk_tiles[t][:, 1, :], 0.0)
        # v in w-major layout [64, 6144] bf16 (transposed into vf chunks)
        vw = singles.tile([64, L], BF, name="vw")
        vf_tiles = [
            singles.tile([128, 65], BF, tag=f"vf{t}", name=f"vf{t}")
            for t in range(NCHUNK)
        ]

        attn_tiles = [
            singles.tile([CK, 512], BF, tag=f"attn{jj}", name=f"attn{jj}")
            for jj in range(8)
        ]
        # x kept resident in bf16: QKV matmul input + stage-4 residual
        xb_tiles = [
            singles.tile([128, 6, 512], BF, tag=f"xb{hb}", name=f"xb{hb}")
            for hb in range(8)
        ]

        def qk_evac(h, src):
            """Copy [64, 192] q|k block (fp32 psum) into the packed fp8
            tiles: q -> group (h*96)//1024 col off, k -> same group at
            1024+off; splits when an h-block crosses a 1024 boundary."""
            p0 = (h * CK) // 1024
            off = h * CK - p0 * 1024

            def one(t, o, cs, cn):
                dst = qk_tiles[t][:, 0, :].rearrange("p (qk n) -> p qk n", qk=2)
                nc.vector.tensor_copy(
                    out=dst[:, :, o:o + cn],
                    in_=src.rearrange("p (qk n) -> p qk n", qk=2)[:, :, cs:cs + cn],
                )

            if off + CK <= 1024:
                one(p0, off, 0, CK)
            else:
                f1 = 1024 - off
                one(p0, off, 0, f1)
                one(p0 + 1, 0, f1, CK - f1)

        # ---- stage 1: QKV in w-major layout ----
        # x loads go through the Pool-triggered SWDGE queue so they never
        # queue behind the 48 vf DMA-transposes on the SP/HWDGE path; the two
        # half tiles + per-k-chunk casts (ScalarE/Pool alternating) unblock
        # the QKV matmuls after ~1/6 of the cast.
        for hb in range(8):  # h in batches of 8
            xch = [
                xpool.tile([128, 3, 512], FP, tag=f"xc{dh}", name=f"xc{hb}_{dh}")
                for dh in range(2)
            ]
            for dh in range(2):
                nc.sync.dma_start(
                    out=xch[dh],
                    in_=x_t[dh * 3:(dh + 1) * 3, :,
                            hb * 512:(hb + 1) * 512].rearrange("k p f -> p k f"),
                )
            xb = xb_tiles[hb]
            for kc in range(6):
                if kc % 2 == 0:
                    nc.scalar.copy(xb[:, kc, :], xch[kc // 3][:, kc % 3, :])
                else:
                    nc.gpsimd.tensor_copy(
                        out=xb[:, kc, :], in_=xch[kc // 3][:, kc % 3, :]
                    )
            for hp in range(4):
                qp = ps_s.tile([128, 3 * CK], FP, tag="s")
                for kc in range(6):
                    nc.tensor.matmul(
                        qp,
                        xb[:, kc, hp * 128:(hp + 1) * 128],
                        wt_sb[:, kc, :],
                        start=(kc == 0),
                        stop=(kc == 5),
                    )
                for sub in range(2):
                    h = hb * 8 + hp * 2 + sub
                    qs = qp[sub * 64:(sub + 1) * 64, :]
                    qk_evac(h, qs[:, 0:2 * CK])
                    eng = nc.scalar.copy if h % 2 else (
                        lambda o, i: nc.vector.tensor_copy(out=o, in_=i)
                    )
                    eng(vw[:, h * CK:(h + 1) * CK], qs[:, 2 * CK:3 * CK])

        # vf chunks via SBUF->SBUF DMA transpose, emitted after all x loads so
        # they never delay stage 1 on the single in-order HWDGE pipe; they
        # drain during stage 2's ramp, just ahead of the PV consumers.
        for t in range(NCHUNK):
            nc.gpsimd.memset(vf_tiles[t][:, 64:65], 1.0)  # denominator ones
            nc.sync.dma_start_transpose(
                out=vf_tiles[t][:, 0:64],
                in_=vw[:, t * 128:(t + 1) * 128],
            )

        # ---- stage 2 + interleaved stage 3/4 ----
        unsb_tiles = {}

        # h-block h (96 attn^T rows) is ready once pass p covers its last
        # column: (h+1)*96 <= 1024*(p+1). Boundary blocks gather from the two
        # SBUF-staged un tiles (usb pool keeps the previous pass alive).
        ready_by_pass = {p: [] for p in range(6)}
        for h in range(64):
            ready_by_pass[((h + 1) * CK - 1) // 1024].append(h)
        jj_after = {p: [] for p in range(6)}
        cnt = [0] * 8
        for p in range(6):
            for h in ready_by_pass[p]:
                cnt[h // 8] += 1
            for jj in range(8):
                if cnt[jj] == 8:
                    jj_after[p].append(jj)
                    cnt[jj] = -(1 << 30)  # emitted

        def emit_h(p, h):
            jj, hl = h // 8, h % 8
            off = h * CK - p * 1024
            if off >= 0:
                src = unsb_tiles[p][:, off:off + CK]
            else:
                f1 = -off  # cols in previous pass
                st = rp.tile([65, CK], FP, tag="st")
                nc.vector.tensor_copy(
                    out=st[:, 0:f1], in_=unsb_tiles[p - 1][:, 1024 - f1:1024]
                )
                nc.vector.tensor_copy(
                    out=st[:, f1:CK], in_=unsb_tiles[p][:, 0:CK - f1]
                )
                src = st
            tp = ps_s.tile([CK, 65], FP, tag="s")
            nc.tensor.transpose(tp, src, ident)
            r = rp.tile([CK, 1], FP, tag="r")
            nc.vector.reciprocal(r, tp[:, 64:65])
            if h % 2 == 0:
                nc.scalar.activation(
                    attn_tiles[jj][:, hl * 64:(hl + 1) * 64], tp[:, 0:64],
                    mybir.ActivationFunctionType.Copy, scale=r,
                )
            else:
                nc.vector.tensor_scalar_mul(
                    attn_tiles[jj][:, hl * 64:(hl + 1) * 64], tp[:, 0:64], r
                )

        ob_tiles = {}

        def emit_co(jj, co):
            if co == 0:
                ob_tiles[jj] = outp.tile(
                    [128, 6, 512], FP, tag="ob", name=f"ob{jj}"
                )
            op = ps_s.tile([128, 512], FP, tag="s")
            nc.tensor.matmul(
                op,
                wct_sb[:, co * 128:(co + 1) * 128],
                attn_tiles[jj],
                start=True,
                stop=True,
            )
            nc.vector.tensor_add(
                ob_tiles[jj][:, co, :], op, xb_tiles[jj][:, co, :]
            )
            if co == 5:
                nc.sync.dma_start(
                    out=out_t[:, :, jj * 512:(jj + 1) * 512].rearrange(
                        "k p f -> p k f"
                    ),
                    in_=ob_tiles[jj],
                )

        # All boundary work (un evacuation, normalization, projection) is
        # injected into the NEXT pass's chunk loop once its S/exp pipeline is
        # rolling; emitting it between passes would block the sp ring and
        # idle both exp engines for several us per boundary.
        def emit_unsb(p, un):
            unsb = usb.tile([65, 1024], FP, tag="unsb")
            unsb_tiles[p] = unsb
            if p % 2 == 0:
                nc.scalar.copy(unsb, un)
            else:
                nc.vector.tensor_copy(out=unsb, in_=un)

        # exp engine per chunk: ScalarE (true Exp) for EXP_ACT of every
        # chunk on average, DVE (Schraudolph) for the rest, interleaved
        inject = []  # [(lt_trigger, fn), ...] boundary work for prev pass
        acc = 0.0
        for p in range(6):
            un = ps_u.tile([65, 1024], FP, tag="un")

            def emit_pv(lt, es, un=un):
                for j in range(2):
                    nc.tensor.matmul(
                        un[:, j * 512:(j + 1) * 512],
                        vf_tiles[lt],
                        es[:, j * 512:(j + 1) * 512],
                        start=(lt == 0),
                        stop=(lt == NCHUNK - 1),
                    )

            # Software-pipelined emission: PE executes its queue in order, so
            # PV(lt) (which waits on exp(lt)) is emitted only after S(lt+2).
            # Depth 2 is required: with depth 1 the stream [PV(lt-1), S(lt)]
            # stalls PE on exp(lt-1), which serializes consecutive exps and
            # idles both exp engines half the time.
            pend = []
            for lt in range(NCHUNK):
                sp = ps_s.tile([128, 1024], FP, tag="s")
                for j in range(2):
                    nc.tensor.matmul(
                        sp[:, j * 512:(j + 1) * 512],
                        qk_tiles[lt // 8][:, :, 1024 + (lt % 8) * 128:1024 + (lt % 8 + 1) * 128],
                        qk_tiles[p][:, :, j * 512:(j + 1) * 512],
                        start=True,
                        stop=True,
                        perf_mode=DR,
                    )
                es = expp.tile([128, 1024], BF, tag="es")
                acc += EXP_ACT
                if acc >= 1.0:
                    acc -= 1.0
                    nc.scalar.activation(
                        es, sp, mybir.ActivationFunctionType.Exp
                    )
                else:
                    nc.vector.tensor_scalar(
                        es[:, :].bitcast(I16), sp, EXP_A, EXP_B, mm, ad
                    )
                pend.append((lt, es))
                if len(pend) > 2:
                    emit_pv(*pend.pop(0))
                for trig, fn in inject:
                    if trig == lt:
                        fn()
            for pe in pend:
                emit_pv(*pe)
            # schedule the boundary work spread across the next pass's chunks
            # (one item every other chunk) so the sp ring always keeps S tiles
            # in flight
            inject = [(0, lambda p=p, un=un: emit_unsb(p, un))]
            slot = 2
            for h in ready_by_pass[p]:
                inject.append((slot, lambda p=p, h=h: emit_h(p, h)))
                slot += 1
            slot += 1
            for jj in jj_after[p]:
                for co in range(6):
                    inject.append(
                        (slot, lambda jj=jj, co=co: emit_co(jj, co))
                    )
                    slot += 2
        for _, fn in inject:
            fn()

    nc.finalize()
    return nc


def _get_program():
    if "nc" not in _CACHE:
        _CACHE["nc"] = _build_program()
    return _CACHE["nc"]


def _host_weights(Wq, Wk, Wv, Wc, gamma):
    wt_host = np.ascontiguousarray(
        np.concatenate([Wq.T, Wk.T, Wv.T], axis=1)
    ).astype(ml_dtypes.bfloat16)                       # [768, 288]
    wct_host = np.ascontiguousarray((gamma[0] * Wc).T).astype(
        ml_dtypes.bfloat16
    )                                                  # [96, 768]
    return wt_host, wct_host


def kernel(x, Wq, Wk, Wv, Wc, gamma):
    x = np.asarray(x, dtype=np.float32)
    Wq = np.asarray(Wq, dtype=np.float32)
    Wk = np.asarray(Wk, dtype=np.float32)
    Wv = np.asarray(Wv, dtype=np.float32)
    Wc = np.asarray(Wc, dtype=np.float32)
    gamma = np.asarray(gamma, dtype=np.float32)

    B = x.shape[0]
    assert x.shape == (B, C, H, W) and B == 8

    wt_host, wct_host = _host_weights(Wq, Wk, Wv, Wc, gamma)
    in_maps = [
        {
            "x": np.ascontiguousarray(x[b].reshape(C, HW)),
            "wt": wt_host,
            "wct": wct_host,
        }
        for b in range(B)
    ]

    nc = _get_program()
    trace = os.environ.get("KERNEL_TRACE", "0") == "1"
    res = run_bass_kernel_spmd(
        nc, in_maps, core_ids=list(range(8)), trace=trace
    )
    if trace and res.exec_time_ns is not None:
        print(f"HW exec time: {res.exec_time_ns} ns")
        _CACHE["exec_time_ns"] = res.exec_time_ns

    out = np.stack([r["out"].reshape(C, H, W) for r in res.results])
    return out
